# revision 1
# baseline (speedup 1.0000x reference)
"""Trainium2 Bass kernel for nn_Net2_54494545051831 (LocallyConnected2d(7x7)
-> bias -> ReLU -> Linear(28392 -> 10)), on 8 NeuronCores.

Distribution: by output location. Each core owns 3 full output rows
(h = 3c .. 3c+2) plus a 6-7 wide piece of rows 24/25 -> 84/85 locations.
Weights / bias / lw are sharded by location (nothing replicated); each core
computes a partial [10, B] of the final linear layer, summed on host.

Per-core compute ("band" layout): for each owned output row, x is reordered
host-side so the contraction rows of location (h, w) sit at band partitions
[32w, 32w+224): band row 32*w' + j = x[:, j//7, h + j%7, w'] for j < 21,
1.0 at j == 21 (bias folds into the weights), 0 above. A location is then
2-3 matmuls with 32-aligned partition windows (zero weights on pad rows).
Two locations run concurrently in the two column halves of the PE array
(M padded 42 -> 64 so the whole [128, 512] psum block relus in one op and
garbage rows are exact zeros). All matmuls bf16 with fp32 accumulation.
"""
import numpy as np
import ml_dtypes

import concourse.mybir as mybir
import concourse.tile as tile
from concourse import bacc
from concourse.bass_utils import run_bass_kernel_spmd

BF16 = mybir.dt.bfloat16
F32 = mybir.dt.float32
RELU = mybir.ActivationFunctionType.Relu

B = 1024
IC, OC, NCLS = 3, 42, 10
KH = KW = 7
OH = OW = 26
NCORES = 8
N_ROWS = 4           # canonical band rows per core (3 full + 1 piece)
STRIDE = 22          # band rows per w'-block: 21 data + 1 ones(bias) row
WINLEN = 6 * STRIDE + 21   # partition span of one location window (153)
TPR = 6              # band tiles per canonical row (704 rows -> 6 tiles)
N_SLOT = 86          # 3*26 + 8 canonical slots (last is always pad)
N_PAIR = 43          # 43 full pairs (86 slots)
NB = 2               # two N-chunks of 512
NCHUNK = 512
MPAD = 64            # output channels padded 42 -> 64

MAX_CHUNKS = 3
WT_COLS = N_SLOT * MAX_CHUNKS * MPAD          # weight sbuf free dim
N_BTILE = N_ROWS * TPR                        # band tiles per core


def _slot_tiles(w_c):
    ta = (STRIDE * w_c) // 128
    tb = (STRIDE * w_c + WINLEN - 1) // 128
    return list(range(ta, tb + 1))

_cache = {}


def _slot_geo(i):
    """Canonical slot -> (band_row, w_c). Rows 0-2 full, row 3 has 8 slots."""
    if i < 78:
        return i // 26, i % 26
    return 3, i - 78


def _build_program():
    if "nc" in _cache:
        return _cache["nc"], _cache["names"]

    nc = bacc.Bacc("TRN2", target_bir_lowering=False, debug=False,
                   num_devices=NCORES)
    band_d = nc.dram_tensor("band", [N_BTILE, 128, B], BF16,
                            kind="ExternalInput").ap()
    wt_d = nc.dram_tensor("wt", [128, WT_COLS], BF16,
                          kind="ExternalInput").ap()
    lwp_d = nc.dram_tensor("lwp", [128, N_PAIR * NCLS], BF16,
                           kind="ExternalInput").ap()
    part_d = nc.dram_tensor("part", [NCLS, B], F32,
                            kind="ExternalOutput").ap()

    with tile.TileContext(nc) as tc:
        with (
            tc.tile_pool(name="sb", bufs=1) as sb,
            tc.tile_pool(name="stk_pool", bufs=10) as stk_pool,
            tc.tile_pool(name="pp_pool", bufs=5, space="PSUM") as pp_pool,
            tc.tile_pool(name="lin_pool", bufs=1, space="PSUM") as lin_pool,
        ):
            band_s = sb.tile([128, N_BTILE * B], BF16)
            wt_s = sb.tile([128, WT_COLS], BF16)
            lwp_s = sb.tile([128, N_PAIR * NCLS], BF16)
            out_s = sb.tile([NCLS, NB * NCHUNK], F32)

            # DMA order: row-0 band tiles + first weight chunk first so
            # the PE can start immediately; rest follows interleaved.
            wchunk = WT_COLS // 8
            def dma_band(t):
                nc.sync.dma_start(
                    out=band_s[:, t * B:(t + 1) * B], in_=band_d[t]
                )
            def dma_wt(d):
                nc.sync.dma_start(
                    out=wt_s[:, d * wchunk:(d + 1) * wchunk],
                    in_=wt_d[:, d * wchunk:(d + 1) * wchunk],
                )
            for t in range(TPR):
                dma_band(t)
            dma_wt(0)
            nc.sync.dma_start(out=lwp_s, in_=lwp_d)
            for r in range(1, N_ROWS):
                for t in range(TPR):
                    dma_band(r * TPR + t)
                for d in range(1 + (r - 1) * 3, min(8, 1 + r * 3)):
                    dma_wt(d)
            for d in range(7, 8):
                dma_wt(d)

            lin_ps = [lin_pool.tile([NCLS, NCHUNK], F32, tag=f"lin{nb}",
                                    name=f"lin_ps{nb}")
                      for nb in range(NB)]

            NBLK = (N_PAIR + 1) // 2
            LIN_LAG = 8          # defer linear MMs by this many pairs

            for nb in range(NB):
                pend = []

                def _flush_lin(limit, nb=nb, pend=None):
                    pass

                def emit_lin(pi, stk, qi, nb=nb):
                    nc.tensor.matmul(
                        lin_ps[nb],
                        lwp_s[:, pi * NCLS:(pi + 1) * NCLS],
                        stk[:, qi * NCHUNK:(qi + 1) * NCHUNK],
                        start=(pi == 0), stop=(pi == N_PAIR - 1),
                        tile_position=(0, 0), skip_group_check=True,
                    )

                for blk in range(N_PAIR):
                    bpairs = [blk]
                    pp = pp_pool.tile([128, 1 * NCHUNK], F32, tag="pp")
                    for qi, pi in enumerate(bpairs):
                        for side in range(2):
                            sl = 2 * pi + side
                            row, w_c = _slot_geo(sl)
                            tiles = _slot_tiles(w_c)
                            nchunks = len(tiles)
                            for ci, tc in enumerate(tiles):
                                btile = row * TPR + tc
                                rhs = band_s[
                                    :,
                                    btile * B + nb * NCHUNK:
                                    btile * B + nb * NCHUNK + NCHUNK,
                                ]
                                lhsT = wt_s[
                                    :,
                                    (sl * MAX_CHUNKS + ci) * MPAD:
                                    (sl * MAX_CHUNKS + ci + 1) * MPAD,
                                ]
                                nc.tensor.matmul(
                                    pp[MPAD * side:MPAD * (side + 1),
                                       qi * NCHUNK:(qi + 1) * NCHUNK],
                                    lhsT, rhs,
                                    start=(ci == 0),
                                    stop=(ci == nchunks - 1),
                                    tile_position=(0, MPAD * side),
                                )
                    stk = stk_pool.tile([128, 2 * NCHUNK], BF16, tag="stk")
                    width = len(bpairs) * NCHUNK
                    if blk % 2 == 0:
                        nc.vector.tensor_scalar_max(
                            stk[:, 0:width], pp[:, 0:width], 0.0)
                    else:
                        nc.scalar.activation(
                            stk[:, 0:width], pp[:, 0:width], RELU)
                    for qi, pi in enumerate(bpairs):
                        pend.append((pi, stk, qi))
                    while len(pend) > LIN_LAG:
                        emit_lin(*pend.pop(0))
                while pend:
                    emit_lin(*pend.pop(0))
                nc.vector.tensor_copy(
                    out_s[:, nb * NCHUNK:(nb + 1) * NCHUNK], lin_ps[nb]
                )
            nc.sync.dma_start(out=part_d, in_=out_s)

    nc.compile()
    names = dict(band="band", wt="wt", lwp="lwp", part="part")
    _cache["nc"] = nc
    _cache["names"] = names
    return nc, names


def _core_slots(c):
    """Actual (h, w) per canonical slot for core c; None = pad."""
    slots = []
    for i in range(78):
        slots.append((3 * c + i // 26, i % 26))
    p0 = (52 * c) // 8
    p1 = (52 * (c + 1)) // 8
    ph, pw0 = 24 + p0 // 26, p0 % 26
    plen = p1 - p0
    for j in range(N_SLOT - 78):
        slots.append((ph, pw0 + j) if j < plen else None)
    return slots, (ph, pw0, plen)


def _prep_core(c, x, W, b, lw4):
    """Build band / wt / lwp arrays for core c."""
    slots, (ph, pw0, plen) = _core_slots(c)

    # bands ------------------------------------------------------------
    # canonical row r: actual output row h_r with block shift s_r
    hs = [(3 * c, 0), (3 * c + 1, 0), (3 * c + 2, 0), (ph, pw0)]
    band = np.zeros((N_BTILE, 128, B), dtype=ml_dtypes.bfloat16)
    cj = np.arange(21) // 7          # channel per j
    kij = np.arange(21) % 7          # kernel-row per j
    for r, (h, shift) in enumerate(hs):
        nblocks = min(32, 32 - shift)
        # blk[j, bw, :] = x[:, cj, h+kij, shift+bw]
        wslice = np.arange(nblocks) + shift
        blk = x[:, cj[:, None], (h + kij)[:, None], wslice[None, :]]
        blk = blk.transpose(1, 2, 0)          # [21, nblocks, B]
        brow = np.zeros((TPR * 128, B), dtype=ml_dtypes.bfloat16)
        for bw in range(nblocks):
            brow[STRIDE * bw:STRIDE * bw + 21] = blk[:, bw]
            brow[STRIDE * bw + 21] = 1.0
        band[r * TPR:(r + 1) * TPR] = brow.reshape(TPR, 128, B)
    # wt ----------------------------------------------------------------
    wt = np.zeros((128, WT_COLS), dtype=ml_dtypes.bfloat16)
    for sl, hw in enumerate(slots):
        if hw is None:
            continue
        h, w = hw
        _, w_c = _slot_geo(sl)
        Wl = W[:, :, h, w, :]                 # [42, 3, 49]
        bl = b[:, h, w]                       # [42]
        for ci, tc in enumerate(_slot_tiles(w_c)):
            col = (sl * MAX_CHUNKS + ci) * MPAD
            rel = 128 * tc + np.arange(128) - STRIDE * w_c
            kj = rel // STRIDE
            jj = rel % STRIDE
            valid = (rel >= 0) & (rel < WINLEN) & (jj < 21)
            vals = np.zeros((128, OC), dtype=np.float32)
            vj, vk = jj[valid], kj[valid]
            vals[valid] = Wl[:, vj // 7, (vj % 7) * 7 + vk].T
            bias_row = (rel >= 0) & (rel < WINLEN) & (jj == 21) & (kj == 0)
            if bias_row.any():
                vals[bias_row] = bl
            wt[:, col:col + OC] = vals.astype(ml_dtypes.bfloat16)
    # lwp ---------------------------------------------------------------
    lwp = np.zeros((128, N_PAIR * NCLS), dtype=ml_dtypes.bfloat16)
    for pi in range(N_PAIR):
        for side in range(2):
            sl = 2 * pi + side
            if slots[sl] is None:
                continue
            h, w = slots[sl]
            lwp[MPAD * side:MPAD * side + OC, pi * NCLS:(pi + 1) * NCLS] = (
                lw4[:, :, h, w].T.astype(ml_dtypes.bfloat16)
            )
    return {"band": band, "wt": wt, "lwp": lwp}


def _run(x, W, b, lw, lb, trace=False):
    nc, _ = _build_program()
    x = np.ascontiguousarray(np.asarray(x, dtype=np.float32))
    W = np.asarray(W, dtype=np.float32)
    b = np.asarray(b, dtype=np.float32)
    lw = np.asarray(lw, dtype=np.float32)
    lb = np.asarray(lb, dtype=np.float32)
    lw4 = lw.reshape(NCLS, OC, OH, OW)
    in_maps = [_prep_core(c, x, W, b, lw4) for c in range(NCORES)]
    res = run_bass_kernel_spmd(
        nc, in_maps, list(range(NCORES)), trace=trace,
    )
    part = np.zeros((NCLS, B), dtype=np.float32)
    for c in range(NCORES):
        part += res.results[c]["part"]
    out = part.T + lb[None, :]
    return out.astype(np.float32), res


def kernel(**inputs):
    out, _ = _run(inputs["x"], inputs["W"], inputs["b"], inputs["lw"],
                  inputs["lb"])
    return out



# revision 4
# speedup vs baseline: 1.4557x; 1.4557x over previous
"""Trainium2 Bass kernel for nn_Net2_54494545051831 (LocallyConnected2d(7x7)
-> bias -> ReLU -> Linear(28392 -> 10)), on 8 NeuronCores.

Distribution: by output location. Each core owns 3 full output rows
(h = 3c .. 3c+2) plus a 6-7 wide piece of rows 24/25 -> 84/85 locations.
Weights / bias / lw are sharded by location (nothing replicated); each core
computes a partial [20, B] of the final linear layer, summed on host.

Per-core compute ("band" layout): for each owned output row, x is reordered
host-side so the contraction rows of location (h, w) sit at band partitions
[22w, 22w+153): band row 22*w' + j = x[:, j//7, h + j%7, w'] for j < 21,
1.0 at j == 21 (bias folds into the weights), 0 above.

Locations are grouped 3 (or 2) per matmul: group weights are [128, 128]
blocks (42 output channels per location side by side), the group's window
union spans 2-3 band tiles -> 2-3 accumulating matmuls of N=512 at full
PE width.  ReLU'd activations for all ~30 groups of a batch-half are kept
in SBUF; the final Linear runs as one batched pass of M=10 matmuls, two at
a time on PE column groups (0,0)/(0,32), accumulating two partial [10,512]
sums that the host adds.  All matmuls bf16 with fp32 accumulation.
"""
import numpy as np
import ml_dtypes

import concourse.mybir as mybir
import concourse.tile as tile
from concourse import bacc
from concourse.bass_utils import run_bass_kernel_spmd

BF16 = mybir.dt.bfloat16
F32 = mybir.dt.float32
RELU = mybir.ActivationFunctionType.Relu

B = 1024
IC, OC, NCLS = 3, 42, 10
KH = KW = 7
OH = OW = 26
NCORES = 8
STRIDE = 22          # band rows per w'-block: 21 data + 1 ones(bias) row
WINLEN = 6 * STRIDE + 21   # partition span of one location window (153)
TPR = 6              # band tiles per full row (704 rows -> 6 tiles)
NB = 2               # two N-chunks of 512
NCHUNK = 512

SPAN = {1: WINLEN, 2: WINLEN + STRIDE, 3: WINLEN + 2 * STRIDE}

# Location groups (start_w, len) per canonical row; DP-optimal for total
# union tiles. Rows 0-2: full 26 locations; row 3: 8 canonical slots.
GROUPS_FULL = [(0, 3), (3, 3), (6, 3), (9, 3), (12, 3), (15, 3),
               (18, 3), (21, 3), (24, 2)]
GROUPS_PART = [(0, 3), (3, 3), (6, 2)]
ROW_TILES = [6, 6, 6, 3]       # band tiles shipped per canonical row
ROW_TBASE = [0, 6, 12, 18]
N_BTILE = sum(ROW_TILES)       # 21


def _group_tiles(w0, L):
    lo = STRIDE * w0
    hi = lo + SPAN[L] - 1
    return list(range(lo // 128, hi // 128 + 1))


def _groups():
    """[(row, w0, L, [band_tile_idx...], chunk_base)]"""
    out = []
    cb = 0
    for r in range(4):
        for (w0, L) in (GROUPS_FULL if r < 3 else GROUPS_PART):
            tiles = [ROW_TBASE[r] + t for t in _group_tiles(w0, L)]
            out.append((r, w0, L, tiles, cb))
            cb += len(tiles)
    return out


GROUPS = _groups()
N_GROUP = len(GROUPS)                       # 30
N_CHUNKS = sum(len(g[3]) for g in GROUPS)   # 73
WT_COLS = N_CHUNKS * 128
N_WARM = 8                                  # PE warm-up dummy matmuls

_cache = {}


def _build_program():
    if "nc" in _cache:
        return _cache["nc"]

    nc = bacc.Bacc("TRN2", target_bir_lowering=False, debug=False,
                   num_devices=NCORES)
    band_d = nc.dram_tensor("band", [N_BTILE, 128, B], BF16,
                            kind="ExternalInput").ap()
    wt_d = nc.dram_tensor("wt", [128, WT_COLS], BF16,
                          kind="ExternalInput").ap()
    lwp_d = nc.dram_tensor("lwp", [128, N_GROUP * NCLS], BF16,
                           kind="ExternalInput").ap()
    part_d = nc.dram_tensor("part", [2 * NCLS, B], F32,
                            kind="ExternalOutput").ap()

    # chunk consumption order == wt column order; group g's chunks are
    # [chunk_base, chunk_base+len) so a plain column-range DMA split works.
    row_last_chunk = {}
    for (r, w0, L, tiles, cb) in GROUPS:
        row_last_chunk[r] = cb + len(tiles)

    with tile.TileContext(nc) as tc:
        with (
            tc.tile_pool(name="sb", bufs=1) as sb,
            tc.tile_pool(name="stk_pool", bufs=N_GROUP) as stk_pool,
            tc.tile_pool(name="pp_pool", bufs=4, space="PSUM") as pp_pool,
            tc.tile_pool(name="lin_pool", bufs=2, space="PSUM") as lin_pool,
            tc.tile_pool(name="warm_pool", bufs=1, space="PSUM") as warm_pool,
        ):
            band_s = sb.tile([128, N_BTILE * B], BF16)
            wt_s = sb.tile([128, WT_COLS], BF16)
            lwp_s = sb.tile([128, N_GROUP * NCLS], BF16)
            out_s = sb.tile([48, NB * NCHUNK], F32)
            warm_s = sb.tile([128, NCHUNK], BF16)

            # --- PE warm-up: self-contained dummy matmuls on a zeroed tile
            # so HAM un-throttles while input DMAs are still in flight.
            nc.gpsimd.memset(warm_s, 0.0)
            warm_ps = warm_pool.tile([128, NCHUNK], F32, name="warm_ps")
            for i in range(N_WARM):
                nc.tensor.matmul(warm_ps, warm_s[:, 0:128], warm_s,
                                 start=True, stop=True,
                                 skip_group_check=True)

            # --- input DMAs, ordered so the first groups' data lands first.
            def dma_band(t, psplit=1):
                for p in range(psplit):
                    pp0, pp1 = 128 * p // psplit, 128 * (p + 1) // psplit
                    nc.sync.dma_start(
                        out=band_s[pp0:pp1, t * B:(t + 1) * B],
                        in_=band_d[t][pp0:pp1],
                    )

            def dma_wt(c0, c1):
                nc.sync.dma_start(
                    out=wt_s[:, c0 * 128:c1 * 128],
                    in_=wt_d[:, c0 * 128:c1 * 128],
                )

            # head: first group's weights (2 chunks) + first two band tiles
            dma_wt(0, GROUPS[0][4] + len(GROUPS[0][3]))
            dma_band(0, psplit=2)
            dma_band(1, psplit=2)
            dma_wt(2, 8)                       # groups 1-2 of row 0
            dma_band(2)
            dma_band(3)
            dma_wt(8, row_last_chunk[0])       # rest of row 0
            dma_band(4)
            dma_band(5)
            for r in range(1, 4):
                for t in range(ROW_TILES[r]):
                    dma_band(ROW_TBASE[r] + t)
                dma_wt(row_last_chunk[r - 1], row_last_chunk[r])
            nc.sync.dma_start(out=lwp_s, in_=lwp_d)

            for nb in range(NB):
                lin_ps = lin_pool.tile([48, NCHUNK], F32, tag="lin",
                                       name=f"lin_ps{nb}")
                stks = []
                for gi, (r, w0, L, tiles, cb) in enumerate(GROUPS):
                    pp = pp_pool.tile([128, NCHUNK], F32, tag="pp")
                    for ci, bt in enumerate(tiles):
                        rhs = band_s[
                            :, bt * B + nb * NCHUNK:
                               bt * B + nb * NCHUNK + NCHUNK]
                        lhsT = wt_s[:, (cb + ci) * 128:(cb + ci + 1) * 128]
                        nc.tensor.matmul(pp, lhsT, rhs,
                                         start=(ci == 0),
                                         stop=(ci == len(tiles) - 1))
                    stk = stk_pool.tile([128, NCHUNK], BF16, tag="stk")
                    if gi % 2 == 0:
                        nc.vector.tensor_scalar_max(stk, pp, 0.0)
                    else:
                        nc.scalar.activation(stk, pp, RELU)
                    stks.append(stk)
                # batched linear pass: two M=10 streams on PE col groups
                n_side = [(N_GROUP + 1) // 2, N_GROUP // 2]
                for gi in range(N_GROUP):
                    side = gi % 2
                    idx = gi // 2
                    nc.tensor.matmul(
                        lin_ps[32 * side:32 * side + NCLS],
                        lwp_s[:, gi * NCLS:(gi + 1) * NCLS],
                        stks[gi],
                        start=(idx == 0), stop=(idx == n_side[side] - 1),
                        tile_position=(0, 32 * side), skip_group_check=True,
                    )
                nc.vector.tensor_copy(
                    out_s[0:NCLS, nb * NCHUNK:(nb + 1) * NCHUNK],
                    lin_ps[0:NCLS])
                nc.vector.tensor_copy(
                    out_s[32:32 + NCLS, nb * NCHUNK:(nb + 1) * NCHUNK],
                    lin_ps[32:32 + NCLS])
                nc.sync.dma_start(
                    out=part_d[0:NCLS, nb * NCHUNK:(nb + 1) * NCHUNK],
                    in_=out_s[0:NCLS, nb * NCHUNK:(nb + 1) * NCHUNK])
                nc.sync.dma_start(
                    out=part_d[NCLS:2 * NCLS, nb * NCHUNK:(nb + 1) * NCHUNK],
                    in_=out_s[32:32 + NCLS, nb * NCHUNK:(nb + 1) * NCHUNK])

    nc.compile()
    _cache["nc"] = nc
    return nc


def _core_rows(c):
    """Per-core geometry: rows 0-2 -> h=3c+r; row 3 -> (ph, pw0, plen)."""
    p0 = (52 * c) // 8
    p1 = (52 * (c + 1)) // 8
    return (24 + p0 // 26, p0 % 26, p1 - p0)


def _row_hw(c, r, w):
    """Canonical (row, local w) -> actual (h, w) or None."""
    if r < 3:
        return (3 * c + r, w)
    ph, pw0, plen = _core_rows(c)
    return (ph, pw0 + w) if w < plen else None


def _prep_core(c, x, W, b, lw4):
    ph, pw0, plen = _core_rows(c)

    # band ---------------------------------------------------------------
    hs = [(3 * c, 0), (3 * c + 1, 0), (3 * c + 2, 0), (ph, pw0)]
    band = np.zeros((N_BTILE, 128, B), dtype=ml_dtypes.bfloat16)
    cj = np.arange(21) // 7          # channel per j
    kij = np.arange(21) % 7          # kernel-row per j
    for r, (h, shift) in enumerate(hs):
        ntile = ROW_TILES[r]
        nblocks = min((ntile * 128) // STRIDE + 1, 32 - shift)
        wslice = np.arange(nblocks) + shift
        blk = x[:, cj[:, None], (h + kij)[:, None], wslice[None, :]]
        blk = blk.transpose(1, 2, 0)          # [21, nblocks, B]
        brow = np.zeros((ntile * 128, B), dtype=ml_dtypes.bfloat16)
        for bw in range(nblocks):
            e = STRIDE * bw + 21
            if e > ntile * 128:
                break
            brow[STRIDE * bw:e] = blk[:, bw]
            if e < ntile * 128:
                brow[e] = 1.0
        band[ROW_TBASE[r]:ROW_TBASE[r] + ntile] = brow.reshape(ntile, 128, B)

    # wt -----------------------------------------------------------------
    wt = np.zeros((128, WT_COLS), dtype=ml_dtypes.bfloat16)
    p = np.arange(128)
    for (r, w0, L, tiles, cb) in GROUPS:
        for ci, bt in enumerate(tiles):
            t_local = bt - ROW_TBASE[r]
            vals = np.zeros((128, 128), dtype=np.float32)
            for li in range(L):
                wl = w0 + li
                hw = _row_hw(c, r, wl)
                if hw is None:
                    continue
                h, w = hw
                rel = 128 * t_local + p - STRIDE * wl
                kj = rel // STRIDE
                jj = rel % STRIDE
                valid = (rel >= 0) & (rel < WINLEN) & (jj < 21)
                vj, vk = jj[valid], kj[valid]
                Wl = W[:, :, h, w, :]                 # [42, 3, 49]
                vals[valid, 42 * li:42 * li + 42] = \
                    Wl[:, vj // 7, (vj % 7) * 7 + vk].T
                bias_row = (rel >= 0) & (rel < WINLEN) & (jj == 21) & (kj == 0)
                if bias_row.any():
                    vals[bias_row, 42 * li:42 * li + 42] = b[:, h, w]
            wt[:, (cb + ci) * 128:(cb + ci + 1) * 128] = \
                vals.astype(ml_dtypes.bfloat16)

    # lwp ----------------------------------------------------------------
    lwp = np.zeros((128, N_GROUP * NCLS), dtype=ml_dtypes.bfloat16)
    for gi, (r, w0, L, tiles, cb) in enumerate(GROUPS):
        for li in range(L):
            hw = _row_hw(c, r, w0 + li)
            if hw is None:
                continue
            h, w = hw
            lwp[42 * li:42 * li + 42, gi * NCLS:(gi + 1) * NCLS] = \
                lw4[:, :, h, w].T.astype(ml_dtypes.bfloat16)
    return {"band": band, "wt": wt, "lwp": lwp}


def _run(x, W, b, lw, lb, trace=False):
    nc = _build_program()
    x = np.ascontiguousarray(np.asarray(x, dtype=np.float32))
    W = np.asarray(W, dtype=np.float32)
    b = np.asarray(b, dtype=np.float32)
    lw = np.asarray(lw, dtype=np.float32)
    lb = np.asarray(lb, dtype=np.float32)
    lw4 = lw.reshape(NCLS, OC, OH, OW)
    in_maps = [_prep_core(c, x, W, b, lw4) for c in range(NCORES)]
    res = run_bass_kernel_spmd(
        nc, in_maps, list(range(NCORES)), trace=trace,
    )
    part = np.zeros((2 * NCLS, B), dtype=np.float32)
    for c in range(NCORES):
        part += res.results[c]["part"]
    out = (part[:NCLS] + part[NCLS:]).T + lb[None, :]
    return out.astype(np.float32), res


def kernel(**inputs):
    out, _ = _run(inputs["x"], inputs["W"], inputs["b"], inputs["lw"],
                  inputs["lb"])
    return out


# revision 6
# speedup vs baseline: 1.7009x; 1.1684x over previous
"""Trainium2 Bass kernel for nn_Net2_54494545051831 (LocallyConnected2d(7x7)
-> bias -> ReLU -> Linear(28392 -> 10)), on 8 NeuronCores.

Distribution: by output location. Each core owns 3 full output rows
(h = 3c .. 3c+2) plus a 6-7 wide piece of rows 24/25 -> 84/85 locations.
Weights / bias / lw are sharded by location (nothing replicated); each core
computes a partial [20, B] of the final linear layer, summed on host.

Per-core compute ("band" layout): for each owned output row, x is reordered
host-side so the contraction rows of location (h, w) sit at band partitions
[22w, 22w+153): band row 22*w' + j = x[:, j//7, h + j%7, w'] for j < 21,
1.0 at j == 21 (bias folds into the weights), 0 above.

Locations are grouped 3 (or 2) per matmul: group weights are [128, 128]
blocks (42 output channels per location side by side), the group's window
union spans 2-3 band tiles -> 2-3 accumulating matmuls of N=512 at full
PE width.  ReLU'd activations for all ~30 groups of a batch-half are kept
in SBUF; the final Linear runs as one batched pass of M=10 matmuls, two at
a time on PE column groups (0,0)/(0,32), accumulating two partial [10,512]
sums that the host adds.  All matmuls bf16 with fp32 accumulation.
"""
import numpy as np
import ml_dtypes

import concourse.mybir as mybir
import concourse.tile as tile
from concourse import bacc
from concourse.bass_utils import run_bass_kernel_spmd

BF16 = mybir.dt.bfloat16
F32 = mybir.dt.float32
RELU = mybir.ActivationFunctionType.Relu

B = 1024
IC, OC, NCLS = 3, 42, 10
KH = KW = 7
OH = OW = 26
NCORES = 8
STRIDE = 22          # band rows per w'-block: 21 data + 1 ones(bias) row
WINLEN = 6 * STRIDE + 21   # partition span of one location window (153)
TPR = 6              # band tiles per full row (704 rows -> 6 tiles)
NB = 2               # two N-chunks of 512
NCHUNK = 512

SPAN = {1: WINLEN, 2: WINLEN + STRIDE, 3: WINLEN + 2 * STRIDE}

# Location groups (start_w, len) per canonical row; DP-optimal for total
# union tiles. Rows 0-2: full 26 locations; row 3: 8 canonical slots.
GROUPS_FULL = [(0, 3), (3, 3), (6, 3), (9, 3), (12, 3), (15, 3),
               (18, 3), (21, 3), (24, 2)]
GROUPS_PART = [(0, 3), (3, 3), (6, 2)]
ROW_TILES = [6, 6, 6, 3]       # band tiles shipped per canonical row
ROW_TBASE = [0, 6, 12, 18]
N_BTILE = sum(ROW_TILES)       # 21


def _group_tiles(w0, L):
    lo = STRIDE * w0
    hi = lo + SPAN[L] - 1
    return list(range(lo // 128, hi // 128 + 1))


def _groups():
    """[(row, w0, L, [band_tile_idx...], chunk_base)]"""
    out = []
    cb = 0
    for r in range(4):
        for (w0, L) in (GROUPS_FULL if r < 3 else GROUPS_PART):
            tiles = [ROW_TBASE[r] + t for t in _group_tiles(w0, L)]
            out.append((r, w0, L, tiles, cb))
            cb += len(tiles)
    return out


GROUPS = _groups()
N_GROUP = len(GROUPS)                       # 30
N_CHUNKS = sum(len(g[3]) for g in GROUPS)   # 73
WT_COLS = N_CHUNKS * 128
N_WARM = 8                                  # PE warm-up dummy matmuls

_cache = {}


def _build_program():
    if "nc" in _cache:
        return _cache["nc"]

    nc = bacc.Bacc("TRN2", target_bir_lowering=False, debug=False,
                   num_devices=NCORES)
    band_d = nc.dram_tensor("band", [N_BTILE, 128, B], BF16,
                            kind="ExternalInput").ap()
    wt_d = nc.dram_tensor("wt", [128, WT_COLS], BF16,
                          kind="ExternalInput").ap()
    lwp_d = nc.dram_tensor("lwp", [128, N_GROUP * NCLS], BF16,
                           kind="ExternalInput").ap()
    part_d = nc.dram_tensor("part", [2 * NCLS, B], F32,
                            kind="ExternalOutput").ap()

    # chunk consumption order == wt column order; group g's chunks are
    # [chunk_base, chunk_base+len) so a plain column-range DMA split works.
    row_last_chunk = {}
    for (r, w0, L, tiles, cb) in GROUPS:
        row_last_chunk[r] = cb + len(tiles)

    with tile.TileContext(nc) as tc:
        with (
            tc.tile_pool(name="sb", bufs=1) as sb,
            tc.tile_pool(name="stk_pool", bufs=NB * N_GROUP) as stk_pool,
            tc.tile_pool(name="pp_pool", bufs=4, space="PSUM") as pp_pool,
            tc.tile_pool(name="lin_pool", bufs=2, space="PSUM") as lin_pool,
            tc.tile_pool(name="warm_pool", bufs=1, space="PSUM") as warm_pool,
        ):
            band_s = sb.tile([128, N_BTILE * B], BF16)
            wt_s = sb.tile([128, WT_COLS], BF16)
            lwp_s = sb.tile([128, N_GROUP * NCLS], BF16)
            out_s = sb.tile([48, NB * NCHUNK], F32)
            warm_s = sb.tile([128, NCHUNK], BF16)

            # --- PE warm-up: self-contained dummy matmuls on a zeroed tile
            # so HAM un-throttles while input DMAs are still in flight.
            nc.gpsimd.memset(warm_s, 0.0)
            warm_ps = warm_pool.tile([128, NCHUNK], F32, name="warm_ps")
            for i in range(N_WARM):
                nc.tensor.matmul(warm_ps, warm_s[:, 0:128], warm_s,
                                 start=True, stop=True,
                                 skip_group_check=True)

            # --- input DMAs, ordered so the first groups' data lands first.
            def dma_band(t, psplit=1):
                for p in range(psplit):
                    pp0, pp1 = 128 * p // psplit, 128 * (p + 1) // psplit
                    nc.sync.dma_start(
                        out=band_s[pp0:pp1, t * B:(t + 1) * B],
                        in_=band_d[t][pp0:pp1],
                    )

            def dma_wt(c0, c1):
                nc.sync.dma_start(
                    out=wt_s[:, c0 * 128:c1 * 128],
                    in_=wt_d[:, c0 * 128:c1 * 128],
                )

            # head: first group's weights (2 chunks) + first two band tiles
            dma_wt(0, GROUPS[0][4] + len(GROUPS[0][3]))
            dma_band(0, psplit=2)
            dma_band(1, psplit=2)
            nc.sync.dma_start(out=lwp_s, in_=lwp_d)
            dma_wt(2, 8)                       # groups 1-2 of row 0
            dma_band(2)
            dma_band(3)
            dma_wt(8, row_last_chunk[0])       # rest of row 0
            dma_band(4)
            dma_band(5)
            for r in range(1, 4):
                for t in range(ROW_TILES[r]):
                    dma_band(ROW_TBASE[r] + t)
                dma_wt(row_last_chunk[r - 1], row_last_chunk[r])

            # LC pass, row-major with both batch-halves per row so PE
            # work per DMA'd byte stays above the DMA delivery rate.
            stks = {}
            for gi, (r, w0, L, tiles, cb) in enumerate(GROUPS):
                for nb in range(NB):
                    pp = pp_pool.tile([128, NCHUNK], F32, tag="pp")
                    for ci, bt in enumerate(tiles):
                        rhs = band_s[
                            :, bt * B + nb * NCHUNK:
                               bt * B + nb * NCHUNK + NCHUNK]
                        lhsT = wt_s[:, (cb + ci) * 128:(cb + ci + 1) * 128]
                        nc.tensor.matmul(pp, lhsT, rhs,
                                         start=(ci == 0),
                                         stop=(ci == len(tiles) - 1))
                    stk = stk_pool.tile([128, NCHUNK], BF16, tag="stk")
                    if (2 * gi + nb) % 2 == 0:
                        nc.vector.tensor_scalar_max(stk, pp, 0.0)
                    else:
                        nc.scalar.activation(stk, pp, RELU)
                    stks[(gi, nb)] = stk

            # batched linear passes: two M=10 streams on PE col groups
            n_side = [(N_GROUP + 1) // 2, N_GROUP // 2]
            for nb in range(NB):
                lin_ps = lin_pool.tile([48, NCHUNK], F32, tag="lin",
                                       name=f"lin_ps{nb}")
                for gi in range(N_GROUP):
                    side = gi % 2
                    idx = gi // 2
                    nc.tensor.matmul(
                        lin_ps[32 * side:32 * side + NCLS],
                        lwp_s[:, gi * NCLS:(gi + 1) * NCLS],
                        stks[(gi, nb)],
                        start=(idx == 0), stop=(idx == n_side[side] - 1),
                        tile_position=(0, 32 * side), skip_group_check=True,
                    )
                nc.vector.tensor_copy(
                    out_s[0:NCLS, nb * NCHUNK:(nb + 1) * NCHUNK],
                    lin_ps[0:NCLS])
                nc.vector.tensor_copy(
                    out_s[32:32 + NCLS, nb * NCHUNK:(nb + 1) * NCHUNK],
                    lin_ps[32:32 + NCLS])
                nc.sync.dma_start(
                    out=part_d[0:NCLS, nb * NCHUNK:(nb + 1) * NCHUNK],
                    in_=out_s[0:NCLS, nb * NCHUNK:(nb + 1) * NCHUNK])
                nc.sync.dma_start(
                    out=part_d[NCLS:2 * NCLS, nb * NCHUNK:(nb + 1) * NCHUNK],
                    in_=out_s[32:32 + NCLS, nb * NCHUNK:(nb + 1) * NCHUNK])

    nc.compile()
    _cache["nc"] = nc
    return nc


def _core_rows(c):
    """Per-core geometry: rows 0-2 -> h=3c+r; row 3 -> (ph, pw0, plen)."""
    p0 = (52 * c) // 8
    p1 = (52 * (c + 1)) // 8
    return (24 + p0 // 26, p0 % 26, p1 - p0)


def _row_hw(c, r, w):
    """Canonical (row, local w) -> actual (h, w) or None."""
    if r < 3:
        return (3 * c + r, w)
    ph, pw0, plen = _core_rows(c)
    return (ph, pw0 + w) if w < plen else None


def _prep_core(c, x, W, b, lw4):
    ph, pw0, plen = _core_rows(c)

    # band ---------------------------------------------------------------
    hs = [(3 * c, 0), (3 * c + 1, 0), (3 * c + 2, 0), (ph, pw0)]
    band = np.zeros((N_BTILE, 128, B), dtype=ml_dtypes.bfloat16)
    cj = np.arange(21) // 7          # channel per j
    kij = np.arange(21) % 7          # kernel-row per j
    for r, (h, shift) in enumerate(hs):
        ntile = ROW_TILES[r]
        nblocks = min((ntile * 128) // STRIDE + 1, 32 - shift)
        wslice = np.arange(nblocks) + shift
        blk = x[:, cj[:, None], (h + kij)[:, None], wslice[None, :]]
        blk = blk.transpose(1, 2, 0)          # [21, nblocks, B]
        brow = np.zeros((ntile * 128, B), dtype=ml_dtypes.bfloat16)
        for bw in range(nblocks):
            e = STRIDE * bw + 21
            if e > ntile * 128:
                break
            brow[STRIDE * bw:e] = blk[:, bw]
            if e < ntile * 128:
                brow[e] = 1.0
        band[ROW_TBASE[r]:ROW_TBASE[r] + ntile] = brow.reshape(ntile, 128, B)

    # wt -----------------------------------------------------------------
    wt = np.zeros((128, WT_COLS), dtype=ml_dtypes.bfloat16)
    p = np.arange(128)
    for (r, w0, L, tiles, cb) in GROUPS:
        for ci, bt in enumerate(tiles):
            t_local = bt - ROW_TBASE[r]
            vals = np.zeros((128, 128), dtype=np.float32)
            for li in range(L):
                wl = w0 + li
                hw = _row_hw(c, r, wl)
                if hw is None:
                    continue
                h, w = hw
                rel = 128 * t_local + p - STRIDE * wl
                kj = rel // STRIDE
                jj = rel % STRIDE
                valid = (rel >= 0) & (rel < WINLEN) & (jj < 21)
                vj, vk = jj[valid], kj[valid]
                Wl = W[:, :, h, w, :]                 # [42, 3, 49]
                vals[valid, 42 * li:42 * li + 42] = \
                    Wl[:, vj // 7, (vj % 7) * 7 + vk].T
                bias_row = (rel >= 0) & (rel < WINLEN) & (jj == 21) & (kj == 0)
                if bias_row.any():
                    vals[bias_row, 42 * li:42 * li + 42] = b[:, h, w]
            wt[:, (cb + ci) * 128:(cb + ci + 1) * 128] = \
                vals.astype(ml_dtypes.bfloat16)

    # lwp ----------------------------------------------------------------
    lwp = np.zeros((128, N_GROUP * NCLS), dtype=ml_dtypes.bfloat16)
    for gi, (r, w0, L, tiles, cb) in enumerate(GROUPS):
        for li in range(L):
            hw = _row_hw(c, r, w0 + li)
            if hw is None:
                continue
            h, w = hw
            lwp[42 * li:42 * li + 42, gi * NCLS:(gi + 1) * NCLS] = \
                lw4[:, :, h, w].T.astype(ml_dtypes.bfloat16)
    return {"band": band, "wt": wt, "lwp": lwp}


def _run(x, W, b, lw, lb, trace=False):
    nc = _build_program()
    x = np.ascontiguousarray(np.asarray(x, dtype=np.float32))
    W = np.asarray(W, dtype=np.float32)
    b = np.asarray(b, dtype=np.float32)
    lw = np.asarray(lw, dtype=np.float32)
    lb = np.asarray(lb, dtype=np.float32)
    lw4 = lw.reshape(NCLS, OC, OH, OW)
    in_maps = [_prep_core(c, x, W, b, lw4) for c in range(NCORES)]
    res = run_bass_kernel_spmd(
        nc, in_maps, list(range(NCORES)), trace=trace,
    )
    part = np.zeros((2 * NCLS, B), dtype=np.float32)
    for c in range(NCORES):
        part += res.results[c]["part"]
    out = (part[:NCLS] + part[NCLS:]).T + lb[None, :]
    return out.astype(np.float32), res


def kernel(**inputs):
    out, _ = _run(inputs["x"], inputs["W"], inputs["b"], inputs["lw"],
                  inputs["lb"])
    return out


# revision 8
# speedup vs baseline: 1.9898x; 1.1698x over previous
"""Trainium2 Bass kernel for nn_Net2_54494545051831 (LocallyConnected2d(7x7)
-> bias -> ReLU -> Linear(28392 -> 10)), on 8 NeuronCores.

Distribution: by output location. Each core owns 3 full output rows
(h = 3c .. 3c+2) plus a 6-7 wide piece of rows 24/25 -> 84/85 locations.
Weights / bias / lw are sharded by location; each core computes two
partial [10, B] pieces of the final linear layer, summed on host.

Per-core layout ("tall band"): for the 3 full output rows, x is reordered
host-side into 28-row blocks per input column w': block row 3*rr + ch =
x[:, ch, 3c + rr, w'] (rr = 0..8 input-row offset), row 27 = 1.0 (bias
folds into the weights).  The contraction for all 3 output rows at one w0
spans rows [28*w0, 28*w0 + 196) - a single group of M = 126 (3 rows x 42
channels) covered by exactly TWO 128-row tiles, using either the natural
tile grid or a 64-row-shifted duplicate grid (both DMA'd) depending on
alignment.  The partial row uses the analogous 22-stride single-row band
with 3 location-groups.  Each group is 2 accumulating matmuls of N=512 at
full PE width; ReLU'd activations for all 58 (group, batch-half) tiles
stay in SBUF; the final Linear runs as batched M=10 matmuls, two at a
time on PE column groups (0,0)/(0,32), accumulating two partial sums the
host adds.  All matmuls bf16 with fp32 accumulation.
"""
import numpy as np
import ml_dtypes

import concourse.mybir as mybir
import concourse.tile as tile
from concourse import bacc
from concourse.bass_utils import run_bass_kernel_spmd

BF16 = mybir.dt.bfloat16
F32 = mybir.dt.float32
RELU = mybir.ActivationFunctionType.Relu

B = 1024
IC, OC, NCLS = 3, 42, 10
KH = KW = 7
OH = OW = 26
NCORES = 8
NB = 2               # two N-chunks of 512
NCHUNK = 512

TS = 28              # tall-band rows per w'-block (27 data + 1 ones)
FSPAN = 6 * TS + 27 + 1    # full-row group window incl bias row (196)
PS = 22              # partial-row band rows per block (21 data + 1 ones)
PWIN = 6 * PS + 21   # single-location window (153)
PSPAN = {2: PWIN + PS, 3: PWIN + 2 * PS}

# band_d tile index map: A0..A6 | B0..B5 | pA0..pA2 | pB0..pB1
N_ATILE, N_BTILE_G, N_PATILE, N_PBTILE = 7, 6, 3, 2
N_BTILE = N_ATILE + N_BTILE_G + N_PATILE + N_PBTILE   # 18
TI_A, TI_B, TI_PA, TI_PB = 0, 7, 13, 16


def _grid_full(w0):
    offA = (TS * w0) % 128
    if offA <= 256 - FSPAN:
        return "A", (TS * w0) // 128
    return "B", (TS * w0 - 64) // 128


def _groups():
    """[(kind, params, tiles, cb)] — kind 'F': params=(w0, grid, k0);
    kind 'P': params=(j0, L, grid, k0). A-grid full groups first."""
    full = [(w0, *_grid_full(w0)) for w0 in range(26)]
    ordered = [f for f in full if f[1] == "A"] + \
              [f for f in full if f[1] == "B"]
    out = []
    cb = 0
    for (w0, g, k0) in ordered:
        base = TI_A if g == "A" else TI_B
        out.append(("F", (w0, g, k0), [base + k0, base + k0 + 1], cb))
        cb += 2
    for (j0, L, g, k0) in [(0, 3, "pA", 0), (6, 2, "pA", 1), (3, 3, "pB", 0)]:
        base = TI_PA if g == "pA" else TI_PB
        out.append(("P", (j0, L, g, k0), [base + k0, base + k0 + 1], cb))
        cb += 2
    return out


GROUPS = _groups()
N_GROUP = len(GROUPS)                       # 29
N_CHUNKS = 2 * N_GROUP                      # 58
WT_COLS = N_CHUNKS * 128
N_WARM = 8                                  # PE warm-up dummy matmuls

_cache = {}


def _build_program():
    if "nc" in _cache:
        return _cache["nc"]

    nc = bacc.Bacc("TRN2", target_bir_lowering=False, debug=False,
                   num_devices=NCORES)
    band_d = nc.dram_tensor("band", [N_BTILE, 128, B], BF16,
                            kind="ExternalInput").ap()
    wt_d = nc.dram_tensor("wt", [128, WT_COLS], BF16,
                          kind="ExternalInput").ap()
    lwp_d = nc.dram_tensor("lwp", [128, N_GROUP * NCLS], BF16,
                           kind="ExternalInput").ap()
    part_d = nc.dram_tensor("part", [2 * NCLS, B], F32,
                            kind="ExternalOutput").ap()

    with tile.TileContext(nc) as tc:
        with (
            tc.tile_pool(name="sb", bufs=1) as sb,
            tc.tile_pool(name="stk_pool", bufs=NB * N_GROUP) as stk_pool,
            tc.tile_pool(name="pp_pool", bufs=4, space="PSUM") as pp_pool,
            tc.tile_pool(name="lin_pool", bufs=2, space="PSUM") as lin_pool,
            tc.tile_pool(name="warm_pool", bufs=1, space="PSUM") as warm_pool,
        ):
            band_s = sb.tile([128, N_BTILE * B], BF16)
            wt_s = sb.tile([128, WT_COLS], BF16)
            lwp_s = sb.tile([128, N_GROUP * NCLS], BF16)
            out_s = sb.tile([48, NB * NCHUNK], F32)
            warm_s = sb.tile([128, NCHUNK], BF16)

            # --- PE warm-up: dummy matmuls on a zeroed tile so HAM
            # un-throttles while the input DMAs are still in flight.
            nc.gpsimd.memset(warm_s, 0.0)
            warm_ps = warm_pool.tile([128, NCHUNK], F32, name="warm_ps")
            for i in range(N_WARM):
                nc.tensor.matmul(warm_ps, warm_s[:, 0:128], warm_s,
                                 start=True, stop=True,
                                 skip_group_check=True)

            # --- input DMAs, interleaved in consumption order.
            def dma_band(t, colsplit=False):
                if colsplit:
                    for nbh in range(NB):
                        nc.sync.dma_start(
                            out=band_s[:, t * B + nbh * NCHUNK:
                                       t * B + (nbh + 1) * NCHUNK],
                            in_=band_d[t][:, nbh * NCHUNK:(nbh + 1) * NCHUNK],
                        )
                else:
                    nc.sync.dma_start(
                        out=band_s[:, t * B:(t + 1) * B], in_=band_d[t])

            def dma_wt(c0, c1):
                nc.sync.dma_start(
                    out=wt_s[:, c0 * 128:c1 * 128],
                    in_=wt_d[:, c0 * 128:c1 * 128],
                )

            dma_wt(0, 4)                 # first two groups' weights
            dma_band(TI_A + 0, colsplit=True)
            dma_band(TI_A + 1, colsplit=True)
            nc.sync.dma_start(out=lwp_s, in_=lwp_d)
            issued = set([TI_A, TI_A + 1])
            wt_hi = 4
            for gi, (kind, params, tiles, cb) in enumerate(GROUPS):
                # stay >=4 groups ahead on weights, in >=12-chunk blocks
                want = cb + 8
                if want > wt_hi and (want - wt_hi >= 12
                                     or want >= N_CHUNKS > wt_hi):
                    hi = min(max(want, wt_hi + 12), N_CHUNKS)
                    dma_wt(wt_hi, hi)
                    wt_hi = hi
                for t in tiles:
                    if t not in issued:
                        dma_band(t)
                        issued.add(t)
            if wt_hi < N_CHUNKS:
                dma_wt(wt_hi, N_CHUNKS)

            # --- LC pass: both batch-halves per group, A-grid groups first.
            stks = {}
            for gi, (kind, params, tiles, cb) in enumerate(GROUPS):
                for nb in range(NB):
                    pp = pp_pool.tile([128, NCHUNK], F32, tag="pp")
                    for ci, bt in enumerate(tiles):
                        rhs = band_s[
                            :, bt * B + nb * NCHUNK:
                               bt * B + nb * NCHUNK + NCHUNK]
                        lhsT = wt_s[:, (cb + ci) * 128:(cb + ci + 1) * 128]
                        nc.tensor.matmul(pp, lhsT, rhs,
                                         start=(ci == 0),
                                         stop=(ci == len(tiles) - 1))
                    stk = stk_pool.tile([128, NCHUNK], BF16, tag="stk")
                    if nb == 0:
                        nc.vector.tensor_scalar_max(stk, pp, 0.0)
                    else:
                        nc.scalar.activation(stk, pp, RELU)
                    stks[(gi, nb)] = stk

            # --- batched linear: two M=10 streams on PE col groups.
            n_side = [(N_GROUP + 1) // 2, N_GROUP // 2]
            for nb in range(NB):
                lin_ps = lin_pool.tile([48, NCHUNK], F32, tag="lin",
                                       name=f"lin_ps{nb}")
                for gi in range(N_GROUP):
                    side = gi % 2
                    idx = gi // 2
                    nc.tensor.matmul(
                        lin_ps[32 * side:32 * side + NCLS],
                        lwp_s[:, gi * NCLS:(gi + 1) * NCLS],
                        stks[(gi, nb)],
                        start=(idx == 0), stop=(idx == n_side[side] - 1),
                        tile_position=(0, 32 * side), skip_group_check=True,
                    )
                nc.vector.tensor_copy(
                    out_s[0:NCLS, nb * NCHUNK:(nb + 1) * NCHUNK],
                    lin_ps[0:NCLS])
                nc.vector.tensor_copy(
                    out_s[32:32 + NCLS, nb * NCHUNK:(nb + 1) * NCHUNK],
                    lin_ps[32:32 + NCLS])
                nc.sync.dma_start(
                    out=part_d[0:NCLS, nb * NCHUNK:(nb + 1) * NCHUNK],
                    in_=out_s[0:NCLS, nb * NCHUNK:(nb + 1) * NCHUNK])
                nc.sync.dma_start(
                    out=part_d[NCLS:2 * NCLS, nb * NCHUNK:(nb + 1) * NCHUNK],
                    in_=out_s[32:32 + NCLS, nb * NCHUNK:(nb + 1) * NCHUNK])

    nc.compile()
    _cache["nc"] = nc
    return nc


def _core_rows(c):
    p0 = (52 * c) // 8
    p1 = (52 * (c + 1)) // 8
    return (24 + p0 // 26, p0 % 26, p1 - p0)


def _prep_core(c, x, W, b, lw4):
    ph, pw0, plen = _core_rows(c)

    # tall band for the 3 full rows ------------------------------------
    fband = np.zeros((N_ATILE * 128, B), dtype=ml_dtypes.bfloat16)
    arr = x[:, :, 3 * c:3 * c + 9, :]          # [B, 3, 9, 32]
    blk = arr.transpose(3, 2, 1, 0).reshape(32, 27, B)   # [w', 3rr+ch, B]
    fb = fband[:32 * TS].reshape(32, TS, B)
    fb[:, :27] = blk.astype(ml_dtypes.bfloat16)
    fb[:, 27] = 1.0

    # 22-stride band for the partial row -------------------------------
    pband = np.zeros(((N_PATILE) * 128, B), dtype=ml_dtypes.bfloat16)
    cj = np.arange(21) // 7
    kij = np.arange(21) % 7
    nblocks = min(14, 32 - pw0)
    wslice = np.arange(nblocks) + pw0
    pblk = x[:, cj[:, None], (ph + kij)[:, None], wslice[None, :]]
    pblk = pblk.transpose(1, 2, 0)             # [21, nblocks, B]
    for bw in range(nblocks):
        e = PS * bw + 21
        pband[PS * bw:e] = pblk[:, bw].astype(ml_dtypes.bfloat16)
        pband[e] = 1.0

    band = np.zeros((N_BTILE, 128, B), dtype=ml_dtypes.bfloat16)
    for k in range(N_ATILE):
        band[TI_A + k] = fband[128 * k:128 * (k + 1)]
    for k in range(N_BTILE_G):
        band[TI_B + k] = fband[64 + 128 * k:64 + 128 * (k + 1)]
    for t in range(N_PATILE):
        band[TI_PA + t] = pband[128 * t:128 * (t + 1)]
    for t in range(N_PBTILE):
        band[TI_PB + t] = pband[64 + 128 * t:64 + 128 * (t + 1)]

    # wt ----------------------------------------------------------------
    wt = np.zeros((128, WT_COLS), dtype=ml_dtypes.bfloat16)
    p = np.arange(128)
    for (kind, params, tiles, cb) in GROUPS:
        for ci in range(2):
            vals = np.zeros((128, 128), dtype=np.float32)
            if kind == "F":
                w0, g, k0 = params
                base = (64 if g == "B" else 0) + 128 * (k0 + ci)
                rel = base + p - TS * w0
                kw = rel // TS
                s = rel % TS
                datav = (rel >= 0) & (rel < FSPAN) & (kw < 7) & (s < 27)
                ch = s % 3
                rr = s // 3
                for r in range(3):
                    kh = rr - r
                    v = datav & (kh >= 0) & (kh < 7)
                    vals[v, 42 * r:42 * r + 42] = \
                        W[:, ch[v], 3 * c + r, w0, kh[v] * 7 + kw[v]].T
                    biasv = (rel >= 0) & (rel < FSPAN) & (kw == 0) & (s == 27)
                    if biasv.any():
                        vals[biasv, 42 * r:42 * r + 42] = b[:, 3 * c + r, w0]
            else:
                j0, L, g, k0 = params
                base = (64 if g == "pB" else 0) + 128 * (k0 + ci)
                for li in range(L):
                    jl = j0 + li
                    if jl >= plen:
                        continue
                    h, w = ph, pw0 + jl
                    rel = base + p - PS * jl
                    kj = rel // PS
                    jj = rel % PS
                    valid = (rel >= 0) & (rel < PWIN) & (jj < 21)
                    vj, vk = jj[valid], kj[valid]
                    Wl = W[:, :, h, w, :]
                    vals[valid, 42 * li:42 * li + 42] = \
                        Wl[:, vj // 7, (vj % 7) * 7 + vk].T
                    biasv = (rel >= 0) & (rel < PWIN) & (jj == 21) & (kj == 0)
                    if biasv.any():
                        vals[biasv, 42 * li:42 * li + 42] = b[:, h, w]
            wt[:, (cb + ci) * 128:(cb + ci + 1) * 128] = \
                vals.astype(ml_dtypes.bfloat16)

    # lwp ----------------------------------------------------------------
    lwp = np.zeros((128, N_GROUP * NCLS), dtype=ml_dtypes.bfloat16)
    for gi, (kind, params, tiles, cb) in enumerate(GROUPS):
        if kind == "F":
            w0 = params[0]
            for r in range(3):
                lwp[42 * r:42 * r + 42, gi * NCLS:(gi + 1) * NCLS] = \
                    lw4[:, :, 3 * c + r, w0].T.astype(ml_dtypes.bfloat16)
        else:
            j0, L = params[0], params[1]
            for li in range(L):
                jl = j0 + li
                if jl >= plen:
                    continue
                lwp[42 * li:42 * li + 42, gi * NCLS:(gi + 1) * NCLS] = \
                    lw4[:, :, ph, pw0 + jl].T.astype(ml_dtypes.bfloat16)
    return {"band": band, "wt": wt, "lwp": lwp}


def _run(x, W, b, lw, lb, trace=False):
    nc = _build_program()
    x = np.ascontiguousarray(np.asarray(x, dtype=np.float32))
    W = np.asarray(W, dtype=np.float32)
    b = np.asarray(b, dtype=np.float32)
    lw = np.asarray(lw, dtype=np.float32)
    lb = np.asarray(lb, dtype=np.float32)
    lw4 = lw.reshape(NCLS, OC, OH, OW)
    in_maps = [_prep_core(c, x, W, b, lw4) for c in range(NCORES)]
    res = run_bass_kernel_spmd(
        nc, in_maps, list(range(NCORES)), trace=trace,
    )
    part = np.zeros((2 * NCLS, B), dtype=np.float32)
    for c in range(NCORES):
        part += res.results[c]["part"]
    out = (part[:NCLS] + part[NCLS:]).T + lb[None, :]
    return out.astype(np.float32), res


def kernel(**inputs):
    out, _ = _run(inputs["x"], inputs["W"], inputs["b"], inputs["lw"],
                  inputs["lb"])
    return out
